# revision 11
# baseline (speedup 1.0000x reference)
"""Self-contained Trainium2 Bass kernel for nn_MinMaxAttention (lightning-style
block-recurrent linear attention with ALiBi decay + RMS norm + gated output
projection).

Sharding: 8 cores = 2 batches x 4 head-groups (4 heads / 512 channels each).
All matmuls run in float32r (fp32 storage, ~1e-4 rel err, full PE rate).
"""
import sys
import math

sys.path.insert(0, '/opt/trn_rl_repo')

import numpy as np
import concourse.bass as bass
import concourse.tile as tile
from concourse import bacc, mybir
from concourse.bass_utils import run_bass_kernel_spmd

F32 = mybir.dt.float32
F32R = mybir.dt.float32r
AF = mybir.ActivationFunctionType

NUM_HEADS = 16
HEAD_DIM = 128
BLOCK = 256
EPS = 1e-6
B_BATCH = 2
N_TOK = 4096
D_IN = 2048
D_OUT = 2048
H_CORE = 4           # heads per core
C_CORE = H_CORE * HEAD_DIM   # hidden channels per core (512)
NB = N_TOK // BLOCK  # 16 attention blocks
KC = D_IN // 128     # 16 contraction chunks
N_CORES = 8
GROUPS = [[0, 1, 2, 3], [4, 5, 6, 7]]


def _get_slopes(n):
    def p2(n):
        start = 2 ** (-2 ** (-(math.log2(n) - 3)))
        return [start * start ** i for i in range(n)]
    if math.log2(n).is_integer():
        return p2(n)
    c = 2 ** math.floor(math.log2(n))
    return p2(c) + _get_slopes(2 * c)[0::2][: n - c]


def build_nc(trace_friendly=False):
    nc = bacc.Bacc("TRN2", target_bir_lowering=False, debug=False,
                   num_devices=N_CORES)

    # ---- I/O ----
    xT_d = nc.dram_tensor("xT", [D_IN, N_TOK], F32R, kind="ExternalInput")
    wq_d = nc.dram_tensor("wq", [D_IN, C_CORE], F32R, kind="ExternalInput")
    wk_d = nc.dram_tensor("wk", [D_IN, C_CORE], F32R, kind="ExternalInput")
    wv_d = nc.dram_tensor("wv", [D_IN, C_CORE], F32R, kind="ExternalInput")
    wg_d = nc.dram_tensor("wg", [D_IN, C_CORE], F32R, kind="ExternalInput")
    wout_d = nc.dram_tensor("wout", [C_CORE, D_OUT], F32R, kind="ExternalInput")
    dmask_d = nc.dram_tensor("dmask", [H_CORE, 2, 128, BLOCK], F32,
                             kind="ExternalInput")
    qdec_d = nc.dram_tensor("qdec", [128, H_CORE, BLOCK], F32R,
                            kind="ExternalInput")
    kdec_d = nc.dram_tensor("kdec", [128, H_CORE, 2], F32, kind="ExternalInput")
    bdec_d = nc.dram_tensor("bdec", [128, H_CORE, 1], F32, kind="ExternalInput")
    ones_d = nc.dram_tensor("ones", [128, 1], F32R, kind="ExternalInput")
    iden_d = nc.dram_tensor("iden", [128, 128], F32R, kind="ExternalInput")
    zer_d = nc.dram_tensor("zer", [128, H_CORE * HEAD_DIM], F32R,
                           kind="ExternalInput")
    out_d = nc.dram_tensor("out", [N_TOK, D_OUT], F32, kind="ExternalOutput")

    with tile.TileContext(nc) as tc:
        with tc.tile_pool(name="dram", bufs=1, space="DRAM") as dram:
            oT_dram = dram.tile([H_CORE, 128, N_TOK], F32)
            gT_dram = dram.tile([H_CORE, 128, N_TOK], F32)
            ssq_local = dram.tile([1, N_TOK], F32)
            ssq_red = dram.tile([1, N_TOK], F32)

            # ===== Phase A: qkv+gate projections + attention (per block) ==
            with (
                tc.tile_pool(name="wpool", bufs=1) as wpool,
                tc.tile_pool(name="cpool", bufs=1) as cpool,
                tc.tile_pool(name="state", bufs=1) as state,
                tc.tile_pool(name="sbA", bufs=2) as sbA,
                tc.tile_pool(name="psP", bufs=1, space="PSUM") as psP,
                tc.tile_pool(name="psA", bufs=1, space="PSUM") as psA,
            ):
                wq_sb = wpool.tile([128, KC, C_CORE], F32R)
                wk_sb = wpool.tile([128, KC, C_CORE], F32R)
                wv_sb = wpool.tile([128, KC, C_CORE], F32R)
                wg_sb = wpool.tile([128, KC, C_CORE], F32R)
                for wsb, wd in ((wq_sb, wq_d), (wk_sb, wk_d),
                                (wv_sb, wv_d), (wg_sb, wg_d)):
                    for k in range(KC):
                        nc.scalar.dma_start(out=wsb[:, k, :],
                                            in_=wd[bass.ts(k, 128), :])

                dmask_sb = cpool.tile([128, H_CORE, 2, BLOCK], F32)
                for h in range(H_CORE):
                    nc.gpsimd.dma_start(out=dmask_sb[:, h, :, :],
                                        in_=dmask_d[h].rearrange("n p m -> p n m"))
                qdec_sb = cpool.tile([128, H_CORE, BLOCK], F32R)
                nc.gpsimd.dma_start(out=qdec_sb[:], in_=qdec_d[:])
                kdec_sb = cpool.tile([128, H_CORE, 2], F32)
                nc.gpsimd.dma_start(out=kdec_sb[:], in_=kdec_d[:])
                bdec_sb = cpool.tile([128, H_CORE, 1], F32)
                nc.gpsimd.dma_start(out=bdec_sb[:], in_=bdec_d[:])
                ones_sb = cpool.tile([128, 1], F32R)
                nc.gpsimd.dma_start(out=ones_sb[:], in_=ones_d[:])
                iden_sb = cpool.tile([128, 128], F32R)
                nc.gpsimd.dma_start(out=iden_sb[:], in_=iden_d[:])

                kv = state.tile([128, H_CORE, HEAD_DIM], F32R)
                nc.sync.dma_start(
                    out=kv[:],
                    in_=zer_d.rearrange("p (h d) -> p h d", h=H_CORE))

                for j in range(NB):
                    tsl = bass.ts(j, BLOCK)
                    xT_blk = sbA.tile([128, KC, BLOCK], F32R, tag="xT")
                    for k in range(KC):
                        nc.sync.dma_start(
                            out=xT_blk[:, k, :],
                            in_=xT_d[bass.ts(k, 128), tsl])

                    qT_s = sbA.tile([128, H_CORE, BLOCK], F32R, tag="qT",
                                    bufs=1)
                    kT_s = sbA.tile([128, H_CORE, BLOCK], F32R, tag="kT",
                                    bufs=1)
                    v_s = sbA.tile([128, 2, C_CORE], F32R, tag="v", bufs=1)

                    # --- projections (transposed q/k; normal v), silu fused
                    for h in range(H_CORE):
                        hsl = bass.ts(h, HEAD_DIM)
                        q_ps = psP.tile([128, BLOCK], F32, tag="proj")
                        for k in range(KC):
                            nc.tensor.matmul(out=q_ps[:],
                                             lhsT=wq_sb[:, k, hsl],
                                             rhs=xT_blk[:, k, :],
                                             start=(k == 0), stop=(k == KC - 1))
                        nc.scalar.activation(out=qT_s[:, h, :], in_=q_ps[:],
                                             func=AF.Silu)
                        k_ps = psP.tile([128, BLOCK], F32, tag="proj")
                        for k in range(KC):
                            nc.tensor.matmul(out=k_ps[:],
                                             lhsT=wk_sb[:, k, hsl],
                                             rhs=xT_blk[:, k, :],
                                             start=(k == 0), stop=(k == KC - 1))
                        nc.scalar.activation(out=kT_s[:, h, :], in_=k_ps[:],
                                             func=AF.Silu)
                    for t2 in range(2):
                        v_ps = psP.tile([128, C_CORE], F32, tag="projv")
                        for k in range(KC):
                            nc.tensor.matmul(out=v_ps[:],
                                             lhsT=xT_blk[:, k, bass.ts(t2, 128)],
                                             rhs=wv_sb[:, k, :],
                                             start=(k == 0), stop=(k == KC - 1))
                        nc.scalar.activation(out=v_s[:, t2, :], in_=v_ps[:],
                                             func=AF.Silu)
                    # --- gate projection, sigmoid fused, spilled
                    for h in range(H_CORE):
                        hsl = bass.ts(h, HEAD_DIM)
                        g_ps = psP.tile([128, BLOCK], F32, tag="proj")
                        for k in range(KC):
                            nc.tensor.matmul(out=g_ps[:],
                                             lhsT=wg_sb[:, k, hsl],
                                             rhs=xT_blk[:, k, :],
                                             start=(k == 0), stop=(k == KC - 1))
                        gt_t = sbA.tile([128, BLOCK], F32, tag="gt")
                        nc.scalar.activation(out=gt_t[:], in_=g_ps[:],
                                             func=AF.Sigmoid)
                        nc.sync.dma_start(out=gT_dram[h, :, tsl], in_=gt_t[:])

                    # --- attention block step per head
                    ssq_ps = psA.tile([1, BLOCK], F32, tag="ssq")
                    for h in range(H_CORE):
                        hsl = bass.ts(h, HEAD_DIM)
                        qsc = sbA.tile([128, BLOCK], F32R, tag="qsc")
                        nc.vector.tensor_mul(qsc[:], qT_s[:, h, :],
                                             qdec_sb[:, h, :])
                        o_ps = psA.tile([128, BLOCK], F32, tag="ops")
                        nc.tensor.matmul(out=o_ps[:], lhsT=kv[:, h, :],
                                         rhs=qsc[:], start=True, stop=False)
                        for n2 in range(2):
                            qk_ps = psA.tile([128, BLOCK], F32, tag="qk")
                            nc.tensor.matmul(out=qk_ps[:],
                                             lhsT=kT_s[:, h, bass.ts(n2, 128)],
                                             rhs=qT_s[:, h, :],
                                             start=True, stop=True)
                            qkm = sbA.tile([128, BLOCK], F32R, tag="qkm")
                            nc.vector.tensor_mul(qkm[:], qk_ps[:],
                                                 dmask_sb[:, h, n2, :])
                            nc.tensor.matmul(out=o_ps[:],
                                             lhsT=v_s[:, n2, hsl],
                                             rhs=qkm[:],
                                             start=False, stop=(n2 == 1))
                        oT_t = sbA.tile([128, BLOCK], F32, tag="oT")
                        nc.vector.tensor_copy(out=oT_t[:], in_=o_ps[:])
                        nc.sync.dma_start(out=oT_dram[h, :, tsl], in_=oT_t[:])
                        sq_t = sbA.tile([128, BLOCK], F32R, tag="sq")
                        nc.vector.tensor_mul(sq_t[:], oT_t[:], oT_t[:])
                        nc.tensor.matmul(out=ssq_ps[:], lhsT=ones_sb[:],
                                         rhs=sq_t[:],
                                         start=(h == 0), stop=(h == H_CORE - 1))
                        kv_ps = psA.tile([128, HEAD_DIM], F32, tag="kvp")
                        for n2 in range(2):
                            kt_ps = psA.tile([128, 128], F32R, tag="ktr")
                            nc.tensor.transpose(kt_ps[:],
                                                kT_s[:, h, bass.ts(n2, 128)],
                                                iden_sb[:])
                            ksc = sbA.tile([128, 128], F32R, tag="ksc")
                            nc.vector.tensor_scalar_mul(
                                ksc[:], kt_ps[:], kdec_sb[:, h, n2:n2 + 1])
                            nc.tensor.matmul(out=kv_ps[:], lhsT=ksc[:],
                                             rhs=v_s[:, n2, hsl],
                                             start=(n2 == 0), stop=(n2 == 1))
                        nc.vector.tensor_scalar_mul(kv[:, h, :], kv[:, h, :],
                                                    bdec_sb[:, h, :])
                        nc.vector.tensor_add(kv[:, h, :], kv[:, h, :],
                                             kv_ps[:])
                    ssq_t = sbA.tile([1, BLOCK], F32, tag="ssqt")
                    nc.vector.tensor_copy(out=ssq_t[:], in_=ssq_ps[:])
                    nc.sync.dma_start(out=ssq_local[0:1, tsl], in_=ssq_t[:])

            # ================= AllReduce of sum-of-squares ================
            nc.gpsimd.collective_compute(
                "AllReduce", mybir.AluOpType.add, replica_groups=GROUPS,
                ins=[ssq_local.opt()], outs=[ssq_red.opt()])

            # ==== Phase C (DVE only): og = oT * sig(g) * rsqrt(var+eps) ===
            # ==== Phase E: partial out projection over local channels  ====
            TB = 512
            NTB = N_TOK // TB
            with (
                tc.tile_pool(name="ogpool", bufs=1) as ogp,
                tc.tile_pool(name="cc", bufs=1) as ccp,
                tc.tile_pool(name="sbC", bufs=2) as sbC,
                tc.tile_pool(name="sbE", bufs=4) as sbE,
                tc.tile_pool(name="psE", bufs=4, space="PSUM") as psE,
            ):
                og_sb = ogp.tile([128, H_CORE, N_TOK], F32R)
                wout_sb = ogp.tile([128, H_CORE, D_OUT], F32R)
                for h in range(H_CORE):
                    nc.sync.dma_start(out=wout_sb[:, h, :],
                                      in_=wout_d[bass.ts(h, 128), :])
                eps_t = ccp.tile([128, 1], F32)
                nc.vector.memset(eps_t[:], EPS)

                for t in range(NTB):
                    tsl = bass.ts(t, TB)
                    inv_t = sbC.tile([128, TB], F32, tag="inv")
                    bc_ap = bass.AP(
                        tensor=ssq_red.opt().tensor,
                        offset=ssq_red.opt().offset + t * TB,
                        ap=[[0, 128], [1, TB]])
                    nc.gpsimd.dma_start(out=inv_t[:], in_=bc_ap)
                    nc.scalar.activation(out=inv_t[:], in_=inv_t[:],
                                         func=AF.Sqrt, scale=1.0 / D_IN,
                                         bias=eps_t[:])
                    nc.vector.reciprocal(out=inv_t[:], in_=inv_t[:])
                    for h in range(H_CORE):
                        oT_t = sbC.tile([128, TB], F32, tag="oTc")
                        nc.gpsimd.dma_start(out=oT_t[:], in_=oT_dram[h, :, tsl])
                        gt_t = sbC.tile([128, TB], F32, tag="gtc")
                        nc.gpsimd.dma_start(out=gt_t[:], in_=gT_dram[h, :, tsl])
                        nc.vector.tensor_mul(og_sb[:, h, tsl], oT_t[:], gt_t[:])
                        nc.vector.tensor_mul(og_sb[:, h, tsl],
                                             og_sb[:, h, tsl], inv_t[:])

                for m in range(N_TOK // 128):
                    msl = bass.ts(m, 128)
                    for oc in range(D_OUT // 512):
                        o_ps = psE.tile([128, 512], F32, tag="out")
                        for h in range(H_CORE):
                            nc.tensor.matmul(
                                out=o_ps[:], lhsT=og_sb[:, h, msl],
                                rhs=wout_sb[:, h, bass.ts(oc, 512)],
                                start=(h == 0), stop=(h == H_CORE - 1))
                        out_t = sbE.tile([128, 512], F32, tag="outT")
                        nc.vector.tensor_copy(out=out_t[:], in_=o_ps[:])
                        nc.sync.dma_start(out=out_d[msl, bass.ts(oc, 512)],
                                          in_=out_t[:])

    nc.compile()
    return nc


_NC_CACHE = {}


def _get_nc():
    if "nc" not in _NC_CACHE:
        _NC_CACHE["nc"] = build_nc()
    return _NC_CACHE["nc"]


def make_in_maps(x, Wqkv, Wg, Wout, norm_w):
    slopes = np.asarray(_get_slopes(NUM_HEADS), dtype=np.float64)
    arr = np.arange(BLOCK, dtype=np.float64) + 1.0
    p_idx = np.arange(128)
    m_idx = np.arange(BLOCK)

    ones = np.ones((128, 1), dtype=np.float32)
    iden = np.eye(128, dtype=np.float32)
    wout_scaled = (np.asarray(norm_w)[:, None] * np.asarray(Wout))

    xT_cache = {}
    in_maps = []
    for c in range(N_CORES):
        bi, hg = c // 4, c % 4
        heads = [hg * H_CORE + i for i in range(H_CORE)]
        if bi not in xT_cache:
            xT_cache[bi] = np.ascontiguousarray(np.asarray(x[bi]).T)
        wq = np.concatenate(
            [Wqkv[:, h * 384:h * 384 + 128] for h in heads], axis=1)
        wk = np.concatenate(
            [Wqkv[:, h * 384 + 128:h * 384 + 256] for h in heads], axis=1)
        wv = np.concatenate(
            [Wqkv[:, h * 384 + 256:h * 384 + 384] for h in heads], axis=1)
        wg = Wg[:, hg * C_CORE:(hg + 1) * C_CORE]
        wout = wout_scaled[hg * C_CORE:(hg + 1) * C_CORE, :]

        dmask = np.zeros((H_CORE, 2, 128, BLOCK), dtype=np.float32)
        qdec = np.zeros((128, H_CORE, BLOCK), dtype=np.float32)
        kdec = np.zeros((128, H_CORE, 2), dtype=np.float32)
        bdec = np.zeros((128, H_CORE, 1), dtype=np.float32)
        for i, h in enumerate(heads):
            s = slopes[h]
            for n2 in range(2):
                n_idx = n2 * 128 + p_idx
                diff = m_idx[None, :] - n_idx[:, None]
                dmask[i, n2] = np.where(
                    diff >= 0, np.exp(-s * diff), 0.0).astype(np.float32)
                kdec[:, i, n2] = np.exp(-s * (BLOCK - (n_idx + 1.0)))
            qdec[:, i, :] = np.exp(-s * arr)[None, :]
            bdec[:, i, 0] = math.exp(-s * BLOCK)

        in_maps.append({
            "xT": np.ascontiguousarray(xT_cache[bi], dtype=np.float32),
            "wq": np.ascontiguousarray(wq, dtype=np.float32),
            "wk": np.ascontiguousarray(wk, dtype=np.float32),
            "wv": np.ascontiguousarray(wv, dtype=np.float32),
            "wg": np.ascontiguousarray(wg, dtype=np.float32),
            "wout": np.ascontiguousarray(wout, dtype=np.float32),
            "dmask": dmask,
            "qdec": qdec,
            "kdec": kdec,
            "bdec": bdec,
            "ones": ones,
            "iden": iden,
            "zer": np.zeros((128, H_CORE * HEAD_DIM), dtype=np.float32),
        })
    return in_maps


def kernel(x, Wqkv, Wg, Wout, norm_w, _trace=False, _trace_kwargs=None):
    x = np.asarray(x)
    in_maps = make_in_maps(np.asarray(x), np.asarray(Wqkv), np.asarray(Wg),
                           np.asarray(Wout), np.asarray(norm_w))
    nc = _get_nc()
    res = run_bass_kernel_spmd(nc, in_maps, list(range(N_CORES)),
                               trace=_trace, **(_trace_kwargs or {}))
    out = np.zeros((B_BATCH, N_TOK, D_OUT), dtype=np.float32)
    for c in range(N_CORES):
        bi = c // 4
        out[bi] += res.results[c]["out"]
    kernel._last_results = res
    return out


# revision 12
# speedup vs baseline: 1.0044x; 1.0044x over previous
"""Self-contained Trainium2 Bass kernel for nn_MinMaxAttention (lightning-style
block-recurrent linear attention with ALiBi decay + RMS norm + gated output
projection).

Sharding: 8 cores = 2 batches x 4 head-groups (4 heads / 512 channels each).
All matmuls run in float32r (fp32 storage, ~1e-4 rel err, full PE rate).
"""
import sys
import math

sys.path.insert(0, '/opt/trn_rl_repo')

import numpy as np
import concourse.bass as bass
import concourse.tile as tile
from concourse import bacc, mybir
from concourse.bass_utils import run_bass_kernel_spmd

F32 = mybir.dt.float32
F32R = mybir.dt.float32r
AF = mybir.ActivationFunctionType

NUM_HEADS = 16
HEAD_DIM = 128
BLOCK = 256
EPS = 1e-6
B_BATCH = 2
N_TOK = 4096
D_IN = 2048
D_OUT = 2048
H_CORE = 4           # heads per core
C_CORE = H_CORE * HEAD_DIM   # hidden channels per core (512)
NB = N_TOK // BLOCK  # 16 attention blocks
KC = D_IN // 128     # 16 contraction chunks
N_CORES = 8
GROUPS = [[0, 1, 2, 3], [4, 5, 6, 7]]


def _get_slopes(n):
    def p2(n):
        start = 2 ** (-2 ** (-(math.log2(n) - 3)))
        return [start * start ** i for i in range(n)]
    if math.log2(n).is_integer():
        return p2(n)
    c = 2 ** math.floor(math.log2(n))
    return p2(c) + _get_slopes(2 * c)[0::2][: n - c]


def build_nc(trace_friendly=False):
    nc = bacc.Bacc("TRN2", target_bir_lowering=False, debug=False,
                   num_devices=N_CORES)

    # ---- I/O ----
    xT_d = nc.dram_tensor("xT", [D_IN, N_TOK], F32R, kind="ExternalInput")
    wq_d = nc.dram_tensor("wq", [D_IN, C_CORE], F32R, kind="ExternalInput")
    wk_d = nc.dram_tensor("wk", [D_IN, C_CORE], F32R, kind="ExternalInput")
    wv_d = nc.dram_tensor("wv", [D_IN, C_CORE], F32R, kind="ExternalInput")
    wg_d = nc.dram_tensor("wg", [D_IN, C_CORE], F32R, kind="ExternalInput")
    wout_d = nc.dram_tensor("wout", [C_CORE, D_OUT], F32R, kind="ExternalInput")
    dmask_d = nc.dram_tensor("dmask", [H_CORE, 2, 128, BLOCK], F32,
                             kind="ExternalInput")
    qdec_d = nc.dram_tensor("qdec", [128, H_CORE, BLOCK], F32R,
                            kind="ExternalInput")
    kdec_d = nc.dram_tensor("kdec", [128, H_CORE, 2], F32, kind="ExternalInput")
    bdec_d = nc.dram_tensor("bdec", [128, H_CORE, 1], F32, kind="ExternalInput")
    ones_d = nc.dram_tensor("ones", [128, 1], F32R, kind="ExternalInput")
    iden_d = nc.dram_tensor("iden", [128, 128], F32R, kind="ExternalInput")
    zer_d = nc.dram_tensor("zer", [128, H_CORE * HEAD_DIM], F32R,
                           kind="ExternalInput")
    out_d = nc.dram_tensor("out", [N_TOK, D_OUT], F32, kind="ExternalOutput")

    with tile.TileContext(nc) as tc:
        with tc.tile_pool(name="dram", bufs=1, space="DRAM") as dram:
            oT_dram = dram.tile([H_CORE, 128, N_TOK], F32)
            gT_dram = dram.tile([H_CORE, 128, N_TOK], F32)
            ssq_local = dram.tile([1, N_TOK], F32)
            ssq_red = dram.tile([1, N_TOK], F32)

            # ===== Phase A: qkv+gate projections + attention (per block) ==
            with (
                tc.tile_pool(name="wpool", bufs=1) as wpool,
                tc.tile_pool(name="cpool", bufs=1) as cpool,
                tc.tile_pool(name="state", bufs=1) as state,
                tc.tile_pool(name="sbA", bufs=2) as sbA,
                tc.tile_pool(name="psP", bufs=1, space="PSUM") as psP,
                tc.tile_pool(name="psA", bufs=1, space="PSUM") as psA,
            ):
                wq_sb = wpool.tile([128, KC, C_CORE], F32R)
                wk_sb = wpool.tile([128, KC, C_CORE], F32R)
                wv_sb = wpool.tile([128, KC, C_CORE], F32R)
                wg_sb = wpool.tile([128, KC, C_CORE], F32R)
                for wsb, wd in ((wq_sb, wq_d), (wk_sb, wk_d),
                                (wv_sb, wv_d), (wg_sb, wg_d)):
                    for k in range(KC):
                        nc.scalar.dma_start(out=wsb[:, k, :],
                                            in_=wd[bass.ts(k, 128), :])

                dmask_sb = cpool.tile([128, H_CORE, 2, BLOCK], F32)
                for h in range(H_CORE):
                    nc.gpsimd.dma_start(out=dmask_sb[:, h, :, :],
                                        in_=dmask_d[h].rearrange("n p m -> p n m"))
                qdec_sb = cpool.tile([128, H_CORE, BLOCK], F32R)
                nc.gpsimd.dma_start(out=qdec_sb[:], in_=qdec_d[:])
                kdec_sb = cpool.tile([128, H_CORE, 2], F32)
                nc.gpsimd.dma_start(out=kdec_sb[:], in_=kdec_d[:])
                bdec_sb = cpool.tile([128, H_CORE, 1], F32)
                nc.gpsimd.dma_start(out=bdec_sb[:], in_=bdec_d[:])
                ones_sb = cpool.tile([128, 1], F32R)
                nc.gpsimd.dma_start(out=ones_sb[:], in_=ones_d[:])
                iden_sb = cpool.tile([128, 128], F32R)
                nc.gpsimd.dma_start(out=iden_sb[:], in_=iden_d[:])

                kv = state.tile([128, H_CORE, HEAD_DIM], F32R)
                nc.sync.dma_start(
                    out=kv[:],
                    in_=zer_d.rearrange("p (h d) -> p h d", h=H_CORE))

                for j in range(NB):
                    tsl = bass.ts(j, BLOCK)
                    xT_blk = sbA.tile([128, KC, BLOCK], F32R, tag="xT")
                    for k in range(KC):
                        nc.sync.dma_start(
                            out=xT_blk[:, k, :],
                            in_=xT_d[bass.ts(k, 128), tsl])

                    qT_s = sbA.tile([128, H_CORE, BLOCK], F32R, tag="qT",
                                    bufs=1)
                    kT_s = sbA.tile([128, H_CORE, BLOCK], F32R, tag="kT",
                                    bufs=1)
                    v_s = sbA.tile([128, 2, C_CORE], F32R, tag="v", bufs=1)

                    # --- projections (transposed q/k; normal v), silu fused
                    for h in range(H_CORE):
                        hsl = bass.ts(h, HEAD_DIM)
                        q_ps = psP.tile([128, BLOCK], F32, tag="proj")
                        for k in range(KC):
                            nc.tensor.matmul(out=q_ps[:],
                                             lhsT=wq_sb[:, k, hsl],
                                             rhs=xT_blk[:, k, :],
                                             start=(k == 0), stop=(k == KC - 1))
                        nc.scalar.activation(out=qT_s[:, h, :], in_=q_ps[:],
                                             func=AF.Silu)
                        k_ps = psP.tile([128, BLOCK], F32, tag="proj")
                        for k in range(KC):
                            nc.tensor.matmul(out=k_ps[:],
                                             lhsT=wk_sb[:, k, hsl],
                                             rhs=xT_blk[:, k, :],
                                             start=(k == 0), stop=(k == KC - 1))
                        nc.scalar.activation(out=kT_s[:, h, :], in_=k_ps[:],
                                             func=AF.Silu)
                    for t2 in range(2):
                        v_ps = psP.tile([128, C_CORE], F32, tag="projv")
                        for k in range(KC):
                            nc.tensor.matmul(out=v_ps[:],
                                             lhsT=xT_blk[:, k, bass.ts(t2, 128)],
                                             rhs=wv_sb[:, k, :],
                                             start=(k == 0), stop=(k == KC - 1))
                        nc.scalar.activation(out=v_s[:, t2, :], in_=v_ps[:],
                                             func=AF.Silu)
                    # --- gate projection, sigmoid fused, spilled
                    for h in range(H_CORE):
                        hsl = bass.ts(h, HEAD_DIM)
                        g_ps = psP.tile([128, BLOCK], F32, tag="proj")
                        for k in range(KC):
                            nc.tensor.matmul(out=g_ps[:],
                                             lhsT=wg_sb[:, k, hsl],
                                             rhs=xT_blk[:, k, :],
                                             start=(k == 0), stop=(k == KC - 1))
                        gt_t = sbA.tile([128, BLOCK], F32, tag="gt")
                        nc.vector.tensor_copy(out=gt_t[:], in_=g_ps[:])
                        nc.sync.dma_start(out=gT_dram[h, :, tsl], in_=gt_t[:])

                    # --- attention block step per head
                    ssq_ps = psA.tile([1, BLOCK], F32, tag="ssq")
                    for h in range(H_CORE):
                        hsl = bass.ts(h, HEAD_DIM)
                        qsc = sbA.tile([128, BLOCK], F32R, tag="qsc")
                        nc.vector.tensor_mul(qsc[:], qT_s[:, h, :],
                                             qdec_sb[:, h, :])
                        o_ps = psA.tile([128, BLOCK], F32, tag="ops")
                        nc.tensor.matmul(out=o_ps[:], lhsT=kv[:, h, :],
                                         rhs=qsc[:], start=True, stop=False)
                        for n2 in range(2):
                            qk_ps = psA.tile([128, BLOCK], F32, tag="qk")
                            nc.tensor.matmul(out=qk_ps[:],
                                             lhsT=kT_s[:, h, bass.ts(n2, 128)],
                                             rhs=qT_s[:, h, :],
                                             start=True, stop=True)
                            qkm = sbA.tile([128, BLOCK], F32R, tag="qkm")
                            nc.vector.tensor_mul(qkm[:], qk_ps[:],
                                                 dmask_sb[:, h, n2, :])
                            nc.tensor.matmul(out=o_ps[:],
                                             lhsT=v_s[:, n2, hsl],
                                             rhs=qkm[:],
                                             start=False, stop=(n2 == 1))
                        oT_t = sbA.tile([128, BLOCK], F32, tag="oT")
                        nc.vector.tensor_copy(out=oT_t[:], in_=o_ps[:])
                        nc.sync.dma_start(out=oT_dram[h, :, tsl], in_=oT_t[:])
                        sq_t = sbA.tile([128, BLOCK], F32R, tag="sq")
                        nc.vector.tensor_mul(sq_t[:], oT_t[:], oT_t[:])
                        nc.tensor.matmul(out=ssq_ps[:], lhsT=ones_sb[:],
                                         rhs=sq_t[:],
                                         start=(h == 0), stop=(h == H_CORE - 1))
                        kv_ps = psA.tile([128, HEAD_DIM], F32, tag="kvp")
                        for n2 in range(2):
                            kt_ps = psA.tile([128, 128], F32R, tag="ktr")
                            nc.tensor.transpose(kt_ps[:],
                                                kT_s[:, h, bass.ts(n2, 128)],
                                                iden_sb[:])
                            ksc = sbA.tile([128, 128], F32R, tag="ksc")
                            nc.vector.tensor_scalar_mul(
                                ksc[:], kt_ps[:], kdec_sb[:, h, n2:n2 + 1])
                            nc.tensor.matmul(out=kv_ps[:], lhsT=ksc[:],
                                             rhs=v_s[:, n2, hsl],
                                             start=(n2 == 0), stop=(n2 == 1))
                        nc.vector.tensor_scalar_mul(kv[:, h, :], kv[:, h, :],
                                                    bdec_sb[:, h, :])
                        nc.vector.tensor_add(kv[:, h, :], kv[:, h, :],
                                             kv_ps[:])
                    ssq_t = sbA.tile([1, BLOCK], F32, tag="ssqt")
                    nc.vector.tensor_copy(out=ssq_t[:], in_=ssq_ps[:])
                    nc.sync.dma_start(out=ssq_local[0:1, tsl], in_=ssq_t[:])

            # ================= AllReduce of sum-of-squares ================
            nc.gpsimd.collective_compute(
                "AllReduce", mybir.AluOpType.add, replica_groups=GROUPS,
                ins=[ssq_local.opt()], outs=[ssq_red.opt()])

            # ==== Phase C (DVE only): og = oT * sig(g) * rsqrt(var+eps) ===
            # ==== Phase E: partial out projection over local channels  ====
            TB = 512
            NTB = N_TOK // TB
            with (
                tc.tile_pool(name="ogpool", bufs=1) as ogp,
                tc.tile_pool(name="cc", bufs=1) as ccp,
                tc.tile_pool(name="sbC", bufs=2) as sbC,
                tc.tile_pool(name="sbE", bufs=4) as sbE,
                tc.tile_pool(name="psE", bufs=4, space="PSUM") as psE,
            ):
                og_sb = ogp.tile([128, H_CORE, N_TOK], F32R)
                wout_sb = ogp.tile([128, H_CORE, D_OUT], F32R)
                for h in range(H_CORE):
                    nc.sync.dma_start(out=wout_sb[:, h, :],
                                      in_=wout_d[bass.ts(h, 128), :])
                eps_t = ccp.tile([128, 1], F32)
                nc.vector.memset(eps_t[:], EPS)

                for t in range(NTB):
                    tsl = bass.ts(t, TB)
                    inv_t = sbC.tile([128, TB], F32, tag="inv")
                    bc_ap = bass.AP(
                        tensor=ssq_red.opt().tensor,
                        offset=ssq_red.opt().offset + t * TB,
                        ap=[[0, 128], [1, TB]])
                    nc.gpsimd.dma_start(out=inv_t[:], in_=bc_ap)
                    nc.scalar.activation(out=inv_t[:], in_=inv_t[:],
                                         func=AF.Sqrt, scale=1.0 / D_IN,
                                         bias=eps_t[:])
                    nc.vector.reciprocal(out=inv_t[:], in_=inv_t[:])
                    for h in range(H_CORE):
                        oT_t = sbC.tile([128, TB], F32, tag="oTc")
                        nc.gpsimd.dma_start(out=oT_t[:], in_=oT_dram[h, :, tsl])
                        gt_t = sbC.tile([128, TB], F32, tag="gtc")
                        nc.gpsimd.dma_start(out=gt_t[:], in_=gT_dram[h, :, tsl])
                        nc.scalar.activation(out=gt_t[:], in_=gt_t[:],
                                             func=AF.Sigmoid)
                        nc.vector.tensor_mul(og_sb[:, h, tsl], oT_t[:], gt_t[:])
                        nc.vector.tensor_mul(og_sb[:, h, tsl],
                                             og_sb[:, h, tsl], inv_t[:])

                for m in range(N_TOK // 128):
                    msl = bass.ts(m, 128)
                    for oc in range(D_OUT // 512):
                        o_ps = psE.tile([128, 512], F32, tag="out")
                        for h in range(H_CORE):
                            nc.tensor.matmul(
                                out=o_ps[:], lhsT=og_sb[:, h, msl],
                                rhs=wout_sb[:, h, bass.ts(oc, 512)],
                                start=(h == 0), stop=(h == H_CORE - 1))
                        out_t = sbE.tile([128, 512], F32, tag="outT")
                        nc.vector.tensor_copy(out=out_t[:], in_=o_ps[:])
                        nc.sync.dma_start(out=out_d[msl, bass.ts(oc, 512)],
                                          in_=out_t[:])

    nc.compile()
    return nc


_NC_CACHE = {}


def _get_nc():
    if "nc" not in _NC_CACHE:
        _NC_CACHE["nc"] = build_nc()
    return _NC_CACHE["nc"]


def make_in_maps(x, Wqkv, Wg, Wout, norm_w):
    slopes = np.asarray(_get_slopes(NUM_HEADS), dtype=np.float64)
    arr = np.arange(BLOCK, dtype=np.float64) + 1.0
    p_idx = np.arange(128)
    m_idx = np.arange(BLOCK)

    ones = np.ones((128, 1), dtype=np.float32)
    iden = np.eye(128, dtype=np.float32)
    wout_scaled = (np.asarray(norm_w)[:, None] * np.asarray(Wout))

    xT_cache = {}
    in_maps = []
    for c in range(N_CORES):
        bi, hg = c // 4, c % 4
        heads = [hg * H_CORE + i for i in range(H_CORE)]
        if bi not in xT_cache:
            xT_cache[bi] = np.ascontiguousarray(np.asarray(x[bi]).T)
        wq = np.concatenate(
            [Wqkv[:, h * 384:h * 384 + 128] for h in heads], axis=1)
        wk = np.concatenate(
            [Wqkv[:, h * 384 + 128:h * 384 + 256] for h in heads], axis=1)
        wv = np.concatenate(
            [Wqkv[:, h * 384 + 256:h * 384 + 384] for h in heads], axis=1)
        wg = Wg[:, hg * C_CORE:(hg + 1) * C_CORE]
        wout = wout_scaled[hg * C_CORE:(hg + 1) * C_CORE, :]

        dmask = np.zeros((H_CORE, 2, 128, BLOCK), dtype=np.float32)
        qdec = np.zeros((128, H_CORE, BLOCK), dtype=np.float32)
        kdec = np.zeros((128, H_CORE, 2), dtype=np.float32)
        bdec = np.zeros((128, H_CORE, 1), dtype=np.float32)
        for i, h in enumerate(heads):
            s = slopes[h]
            for n2 in range(2):
                n_idx = n2 * 128 + p_idx
                diff = m_idx[None, :] - n_idx[:, None]
                dmask[i, n2] = np.where(
                    diff >= 0, np.exp(-s * diff), 0.0).astype(np.float32)
                kdec[:, i, n2] = np.exp(-s * (BLOCK - (n_idx + 1.0)))
            qdec[:, i, :] = np.exp(-s * arr)[None, :]
            bdec[:, i, 0] = math.exp(-s * BLOCK)

        in_maps.append({
            "xT": np.ascontiguousarray(xT_cache[bi], dtype=np.float32),
            "wq": np.ascontiguousarray(wq, dtype=np.float32),
            "wk": np.ascontiguousarray(wk, dtype=np.float32),
            "wv": np.ascontiguousarray(wv, dtype=np.float32),
            "wg": np.ascontiguousarray(wg, dtype=np.float32),
            "wout": np.ascontiguousarray(wout, dtype=np.float32),
            "dmask": dmask,
            "qdec": qdec,
            "kdec": kdec,
            "bdec": bdec,
            "ones": ones,
            "iden": iden,
            "zer": np.zeros((128, H_CORE * HEAD_DIM), dtype=np.float32),
        })
    return in_maps


def kernel(x, Wqkv, Wg, Wout, norm_w, _trace=False, _trace_kwargs=None):
    x = np.asarray(x)
    in_maps = make_in_maps(np.asarray(x), np.asarray(Wqkv), np.asarray(Wg),
                           np.asarray(Wout), np.asarray(norm_w))
    nc = _get_nc()
    res = run_bass_kernel_spmd(nc, in_maps, list(range(N_CORES)),
                               trace=_trace, **(_trace_kwargs or {}))
    out = np.zeros((B_BATCH, N_TOK, D_OUT), dtype=np.float32)
    for c in range(N_CORES):
        bi = c // 4
        out[bi] += res.results[c]["out"]
    kernel._last_results = res
    return out


# revision 14
# speedup vs baseline: 1.0275x; 1.0230x over previous
"""Self-contained Trainium2 Bass kernel for nn_MinMaxAttention (lightning-style
block-recurrent linear attention with ALiBi decay + RMS norm + gated output
projection).

Sharding: 8 cores = 2 batches x 4 head-groups (4 heads / 512 channels each).
All matmuls run in float32r (fp32 storage, ~1e-4 rel err, full PE rate).
"""
import sys
import math

sys.path.insert(0, '/opt/trn_rl_repo')

import numpy as np
import concourse.bass as bass
import concourse.tile as tile
from concourse import bacc, mybir
from concourse.bass_utils import run_bass_kernel_spmd

F32 = mybir.dt.float32
F32R = mybir.dt.float32r
AF = mybir.ActivationFunctionType

NUM_HEADS = 16
HEAD_DIM = 128
BLOCK = 256
EPS = 1e-6
B_BATCH = 2
N_TOK = 4096
D_IN = 2048
D_OUT = 2048
H_CORE = 4           # heads per core
C_CORE = H_CORE * HEAD_DIM   # hidden channels per core (512)
NB = N_TOK // BLOCK  # 16 attention blocks
KC = D_IN // 128     # 16 contraction chunks
N_CORES = 8
GROUPS = [[0, 1, 2, 3], [4, 5, 6, 7]]


def _get_slopes(n):
    def p2(n):
        start = 2 ** (-2 ** (-(math.log2(n) - 3)))
        return [start * start ** i for i in range(n)]
    if math.log2(n).is_integer():
        return p2(n)
    c = 2 ** math.floor(math.log2(n))
    return p2(c) + _get_slopes(2 * c)[0::2][: n - c]


def build_nc(trace_friendly=False):
    nc = bacc.Bacc("TRN2", target_bir_lowering=False, debug=False,
                   num_devices=N_CORES)

    # ---- I/O ----
    xT_d = nc.dram_tensor("xT", [D_IN, N_TOK], F32R, kind="ExternalInput")
    wq_d = nc.dram_tensor("wq", [D_IN, C_CORE], F32R, kind="ExternalInput")
    wk_d = nc.dram_tensor("wk", [D_IN, C_CORE], F32R, kind="ExternalInput")
    wv_d = nc.dram_tensor("wv", [D_IN, C_CORE], F32R, kind="ExternalInput")
    wg_d = nc.dram_tensor("wg", [D_IN, C_CORE], F32R, kind="ExternalInput")
    wout_d = nc.dram_tensor("wout", [C_CORE, D_OUT], F32R, kind="ExternalInput")
    dmask_d = nc.dram_tensor("dmask", [H_CORE, 2, 128, BLOCK], F32,
                             kind="ExternalInput")
    qdec_d = nc.dram_tensor("qdec", [128, H_CORE, BLOCK], F32R,
                            kind="ExternalInput")
    kdec_d = nc.dram_tensor("kdec", [128, H_CORE, 2], F32, kind="ExternalInput")
    bdec_d = nc.dram_tensor("bdec", [128, H_CORE, 1], F32, kind="ExternalInput")
    ones_d = nc.dram_tensor("ones", [128, 1], F32R, kind="ExternalInput")
    iden_d = nc.dram_tensor("iden", [128, 128], F32R, kind="ExternalInput")
    zer_d = nc.dram_tensor("zer", [128, H_CORE * HEAD_DIM], F32R,
                           kind="ExternalInput")
    out_d = nc.dram_tensor("out", [N_TOK, D_OUT], F32, kind="ExternalOutput")

    with tile.TileContext(nc) as tc:
        with tc.tile_pool(name="dram", bufs=1, space="DRAM") as dram:
            oT_dram = dram.tile([H_CORE, 128, N_TOK], F32)
            gT_dram = dram.tile([H_CORE, 128, N_TOK], F32)
            ssq_local = dram.tile([1, N_TOK], F32)
            ssq_red = dram.tile([1, N_TOK], F32)

            # ===== Phase A: qkv+gate projections + attention (per block) ==
            with (
                tc.tile_pool(name="wpool", bufs=1) as wpool,
                tc.tile_pool(name="cpool", bufs=1) as cpool,
                tc.tile_pool(name="state", bufs=1) as state,
                tc.tile_pool(name="sbA", bufs=2) as sbA,
                tc.tile_pool(name="psP", bufs=1, space="PSUM") as psP,
                tc.tile_pool(name="psA", bufs=1, space="PSUM") as psA,
            ):
                wq_sb = wpool.tile([128, KC, C_CORE], F32R)
                wk_sb = wpool.tile([128, KC, C_CORE], F32R)
                wv_sb = wpool.tile([128, KC, C_CORE], F32R)
                wg_sb = wpool.tile([128, KC, C_CORE], F32R)
                for wsb, wd in ((wq_sb, wq_d), (wk_sb, wk_d),
                                (wv_sb, wv_d), (wg_sb, wg_d)):
                    for k in range(KC):
                        nc.scalar.dma_start(out=wsb[:, k, :],
                                            in_=wd[bass.ts(k, 128), :])

                dmask_sb = cpool.tile([128, H_CORE, 2, BLOCK], F32)
                for h in range(H_CORE):
                    nc.gpsimd.dma_start(out=dmask_sb[:, h, :, :],
                                        in_=dmask_d[h].rearrange("n p m -> p n m"))
                qdec_sb = cpool.tile([128, H_CORE, BLOCK], F32R)
                nc.gpsimd.dma_start(out=qdec_sb[:], in_=qdec_d[:])
                kdec_sb = cpool.tile([128, H_CORE, 2], F32)
                nc.gpsimd.dma_start(out=kdec_sb[:], in_=kdec_d[:])
                bdec_sb = cpool.tile([128, H_CORE, 1], F32)
                nc.gpsimd.dma_start(out=bdec_sb[:], in_=bdec_d[:])
                ones_sb = cpool.tile([128, 1], F32R)
                nc.gpsimd.dma_start(out=ones_sb[:], in_=ones_d[:])
                iden_sb = cpool.tile([128, 128], F32R)
                nc.gpsimd.dma_start(out=iden_sb[:], in_=iden_d[:])

                kv = state.tile([128, H_CORE, HEAD_DIM], F32R)
                nc.sync.dma_start(
                    out=kv[:],
                    in_=zer_d.rearrange("p (h d) -> p h d", h=H_CORE))

                for j in range(NB):
                    tsl = bass.ts(j, BLOCK)
                    xT_blk = sbA.tile([128, KC, BLOCK], F32R, tag="xT")
                    for k in range(KC):
                        nc.sync.dma_start(
                            out=xT_blk[:, k, :],
                            in_=xT_d[bass.ts(k, 128), tsl])

                    qT_s = sbA.tile([128, H_CORE, BLOCK], F32R, tag="qT",
                                    bufs=1)
                    kT_s = sbA.tile([128, H_CORE, BLOCK], F32R, tag="kT",
                                    bufs=1)
                    v_s = sbA.tile([128, 2, C_CORE], F32R, tag="v", bufs=1)

                    # --- projections (transposed q/k; normal v), silu fused
                    for h in range(H_CORE):
                        hsl = bass.ts(h, HEAD_DIM)
                        q_ps = psP.tile([128, BLOCK], F32, tag="proj", bufs=2)
                        for k in range(KC):
                            nc.tensor.matmul(out=q_ps[:],
                                             lhsT=wq_sb[:, k, hsl],
                                             rhs=xT_blk[:, k, :],
                                             start=(k == 0), stop=(k == KC - 1))
                        nc.scalar.activation(out=qT_s[:, h, :], in_=q_ps[:],
                                             func=AF.Silu)
                        k_ps = psP.tile([128, BLOCK], F32, tag="proj", bufs=2)
                        for k in range(KC):
                            nc.tensor.matmul(out=k_ps[:],
                                             lhsT=wk_sb[:, k, hsl],
                                             rhs=xT_blk[:, k, :],
                                             start=(k == 0), stop=(k == KC - 1))
                        nc.scalar.activation(out=kT_s[:, h, :], in_=k_ps[:],
                                             func=AF.Silu)
                    for t2 in range(2):
                        v_ps = psP.tile([128, C_CORE], F32, tag="projv")
                        for k in range(KC):
                            nc.tensor.matmul(out=v_ps[:],
                                             lhsT=xT_blk[:, k, bass.ts(t2, 128)],
                                             rhs=wv_sb[:, k, :],
                                             start=(k == 0), stop=(k == KC - 1))
                        nc.scalar.activation(out=v_s[:, t2, :], in_=v_ps[:],
                                             func=AF.Silu)
                    # --- gate projection, sigmoid fused, spilled
                    for h in range(H_CORE):
                        hsl = bass.ts(h, HEAD_DIM)
                        g_ps = psP.tile([128, BLOCK], F32, tag="proj", bufs=2)
                        for k in range(KC):
                            nc.tensor.matmul(out=g_ps[:],
                                             lhsT=wg_sb[:, k, hsl],
                                             rhs=xT_blk[:, k, :],
                                             start=(k == 0), stop=(k == KC - 1))
                        gt_t = sbA.tile([128, BLOCK], F32, tag="gt")
                        nc.vector.tensor_copy(out=gt_t[:], in_=g_ps[:])
                        nc.sync.dma_start(out=gT_dram[h, :, tsl], in_=gt_t[:])

                    # --- attention block step per head
                    ssq_ps = psA.tile([1, BLOCK], F32, tag="ssq")
                    for h in range(H_CORE):
                        hsl = bass.ts(h, HEAD_DIM)
                        qsc = sbA.tile([128, BLOCK], F32R, tag="qsc")
                        nc.vector.tensor_mul(qsc[:], qT_s[:, h, :],
                                             qdec_sb[:, h, :])
                        o_ps = psA.tile([128, BLOCK], F32, tag="ops", bufs=2)
                        nc.tensor.matmul(out=o_ps[:], lhsT=kv[:, h, :],
                                         rhs=qsc[:], start=True, stop=False)
                        for n2 in range(2):
                            qk_ps = psA.tile([128, BLOCK], F32, tag="qk", bufs=2)
                            nc.tensor.matmul(out=qk_ps[:],
                                             lhsT=kT_s[:, h, bass.ts(n2, 128)],
                                             rhs=qT_s[:, h, :],
                                             start=True, stop=True)
                            qkm = sbA.tile([128, BLOCK], F32R, tag="qkm")
                            nc.vector.tensor_mul(qkm[:], qk_ps[:],
                                                 dmask_sb[:, h, n2, :])
                            nc.tensor.matmul(out=o_ps[:],
                                             lhsT=v_s[:, n2, hsl],
                                             rhs=qkm[:],
                                             start=False, stop=(n2 == 1))
                        oT_t = sbA.tile([128, BLOCK], F32, tag="oT")
                        nc.vector.tensor_copy(out=oT_t[:], in_=o_ps[:])
                        nc.sync.dma_start(out=oT_dram[h, :, tsl], in_=oT_t[:])
                        sq_t = sbA.tile([128, BLOCK], F32R, tag="sq")
                        nc.vector.tensor_mul(sq_t[:], oT_t[:], oT_t[:])
                        nc.tensor.matmul(out=ssq_ps[:], lhsT=ones_sb[:],
                                         rhs=sq_t[:],
                                         start=(h == 0), stop=(h == H_CORE - 1))
                        kv_ps = psA.tile([128, BLOCK], F32, tag="ops", bufs=2)
                        for n2 in range(2):
                            kt_ps = psA.tile([128, BLOCK], F32R, tag="qk", bufs=2)
                            nc.tensor.transpose(kt_ps[:, 0:128],
                                                kT_s[:, h, bass.ts(n2, 128)],
                                                iden_sb[:])
                            ksc = sbA.tile([128, 128], F32R, tag="ksc")
                            nc.vector.tensor_scalar_mul(
                                ksc[:], kt_ps[:, 0:128], kdec_sb[:, h, n2:n2 + 1])
                            nc.tensor.matmul(out=kv_ps[:, 0:128], lhsT=ksc[:],
                                             rhs=v_s[:, n2, hsl],
                                             start=(n2 == 0), stop=(n2 == 1))
                        nc.vector.tensor_scalar_mul(kv[:, h, :], kv[:, h, :],
                                                    bdec_sb[:, h, :])
                        nc.vector.tensor_add(kv[:, h, :], kv[:, h, :],
                                             kv_ps[:, 0:128])
                    ssq_t = sbA.tile([1, BLOCK], F32, tag="ssqt")
                    nc.vector.tensor_copy(out=ssq_t[:], in_=ssq_ps[:])
                    nc.sync.dma_start(out=ssq_local[0:1, tsl], in_=ssq_t[:])

            # ================= AllReduce of sum-of-squares ================
            nc.gpsimd.collective_compute(
                "AllReduce", mybir.AluOpType.add, replica_groups=GROUPS,
                ins=[ssq_local.opt()], outs=[ssq_red.opt()])

            # ==== Phase C (DVE only): og = oT * sig(g) * rsqrt(var+eps) ===
            # ==== Phase E: partial out projection over local channels  ====
            TB = 512
            NTB = N_TOK // TB
            with (
                tc.tile_pool(name="ogpool", bufs=1) as ogp,
                tc.tile_pool(name="cc", bufs=1) as ccp,
                tc.tile_pool(name="sbC", bufs=2) as sbC,
                tc.tile_pool(name="sbE", bufs=4) as sbE,
                tc.tile_pool(name="psE", bufs=4, space="PSUM") as psE,
            ):
                og_sb = ogp.tile([128, H_CORE, N_TOK], F32R)
                wout_sb = ogp.tile([128, H_CORE, D_OUT], F32R)
                for h in range(H_CORE):
                    nc.sync.dma_start(out=wout_sb[:, h, :],
                                      in_=wout_d[bass.ts(h, 128), :])
                eps_t = ccp.tile([128, 1], F32)
                nc.vector.memset(eps_t[:], EPS)

                for t in range(NTB):
                    tsl = bass.ts(t, TB)
                    inv_t = sbC.tile([128, TB], F32, tag="inv")
                    bc_ap = bass.AP(
                        tensor=ssq_red.opt().tensor,
                        offset=ssq_red.opt().offset + t * TB,
                        ap=[[0, 128], [1, TB]])
                    nc.gpsimd.dma_start(out=inv_t[:], in_=bc_ap)
                    nc.scalar.activation(out=inv_t[:], in_=inv_t[:],
                                         func=AF.Sqrt, scale=1.0 / D_IN,
                                         bias=eps_t[:])
                    nc.vector.reciprocal(out=inv_t[:], in_=inv_t[:])
                    for h in range(H_CORE):
                        oT_t = sbC.tile([128, TB], F32, tag="oTc")
                        nc.gpsimd.dma_start(out=oT_t[:], in_=oT_dram[h, :, tsl])
                        gt_t = sbC.tile([128, TB], F32, tag="gtc")
                        nc.gpsimd.dma_start(out=gt_t[:], in_=gT_dram[h, :, tsl])
                        nc.scalar.activation(out=gt_t[:], in_=gt_t[:],
                                             func=AF.Sigmoid)
                        nc.vector.tensor_mul(og_sb[:, h, tsl], oT_t[:], gt_t[:])
                        nc.vector.tensor_mul(og_sb[:, h, tsl],
                                             og_sb[:, h, tsl], inv_t[:])

                for m in range(N_TOK // 128):
                    msl = bass.ts(m, 128)
                    for oc in range(D_OUT // 512):
                        o_ps = psE.tile([128, 512], F32, tag="out")
                        for h in range(H_CORE):
                            nc.tensor.matmul(
                                out=o_ps[:], lhsT=og_sb[:, h, msl],
                                rhs=wout_sb[:, h, bass.ts(oc, 512)],
                                start=(h == 0), stop=(h == H_CORE - 1))
                        out_t = sbE.tile([128, 512], F32, tag="outT")
                        nc.vector.tensor_copy(out=out_t[:], in_=o_ps[:])
                        nc.sync.dma_start(out=out_d[msl, bass.ts(oc, 512)],
                                          in_=out_t[:])

    nc.compile()
    return nc


_NC_CACHE = {}


def _get_nc():
    if "nc" not in _NC_CACHE:
        _NC_CACHE["nc"] = build_nc()
    return _NC_CACHE["nc"]


def make_in_maps(x, Wqkv, Wg, Wout, norm_w):
    slopes = np.asarray(_get_slopes(NUM_HEADS), dtype=np.float64)
    arr = np.arange(BLOCK, dtype=np.float64) + 1.0
    p_idx = np.arange(128)
    m_idx = np.arange(BLOCK)

    ones = np.ones((128, 1), dtype=np.float32)
    iden = np.eye(128, dtype=np.float32)
    wout_scaled = (np.asarray(norm_w)[:, None] * np.asarray(Wout))

    xT_cache = {}
    in_maps = []
    for c in range(N_CORES):
        bi, hg = c // 4, c % 4
        heads = [hg * H_CORE + i for i in range(H_CORE)]
        if bi not in xT_cache:
            xT_cache[bi] = np.ascontiguousarray(np.asarray(x[bi]).T)
        wq = np.concatenate(
            [Wqkv[:, h * 384:h * 384 + 128] for h in heads], axis=1)
        wk = np.concatenate(
            [Wqkv[:, h * 384 + 128:h * 384 + 256] for h in heads], axis=1)
        wv = np.concatenate(
            [Wqkv[:, h * 384 + 256:h * 384 + 384] for h in heads], axis=1)
        wg = Wg[:, hg * C_CORE:(hg + 1) * C_CORE]
        wout = wout_scaled[hg * C_CORE:(hg + 1) * C_CORE, :]

        dmask = np.zeros((H_CORE, 2, 128, BLOCK), dtype=np.float32)
        qdec = np.zeros((128, H_CORE, BLOCK), dtype=np.float32)
        kdec = np.zeros((128, H_CORE, 2), dtype=np.float32)
        bdec = np.zeros((128, H_CORE, 1), dtype=np.float32)
        for i, h in enumerate(heads):
            s = slopes[h]
            for n2 in range(2):
                n_idx = n2 * 128 + p_idx
                diff = m_idx[None, :] - n_idx[:, None]
                dmask[i, n2] = np.where(
                    diff >= 0, np.exp(-s * diff), 0.0).astype(np.float32)
                kdec[:, i, n2] = np.exp(-s * (BLOCK - (n_idx + 1.0)))
            qdec[:, i, :] = np.exp(-s * arr)[None, :]
            bdec[:, i, 0] = math.exp(-s * BLOCK)

        in_maps.append({
            "xT": np.ascontiguousarray(xT_cache[bi], dtype=np.float32),
            "wq": np.ascontiguousarray(wq, dtype=np.float32),
            "wk": np.ascontiguousarray(wk, dtype=np.float32),
            "wv": np.ascontiguousarray(wv, dtype=np.float32),
            "wg": np.ascontiguousarray(wg, dtype=np.float32),
            "wout": np.ascontiguousarray(wout, dtype=np.float32),
            "dmask": dmask,
            "qdec": qdec,
            "kdec": kdec,
            "bdec": bdec,
            "ones": ones,
            "iden": iden,
            "zer": np.zeros((128, H_CORE * HEAD_DIM), dtype=np.float32),
        })
    return in_maps


def kernel(x, Wqkv, Wg, Wout, norm_w, _trace=False, _trace_kwargs=None):
    x = np.asarray(x)
    in_maps = make_in_maps(np.asarray(x), np.asarray(Wqkv), np.asarray(Wg),
                           np.asarray(Wout), np.asarray(norm_w))
    nc = _get_nc()
    res = run_bass_kernel_spmd(nc, in_maps, list(range(N_CORES)),
                               trace=_trace, **(_trace_kwargs or {}))
    out = np.zeros((B_BATCH, N_TOK, D_OUT), dtype=np.float32)
    for c in range(N_CORES):
        bi = c // 4
        out[bi] += res.results[c]["out"]
    kernel._last_results = res
    return out
